# revision 1
# baseline (speedup 1.0000x reference)
"""AKT-style dense transformer (distance-decay attention) on 8 Trainium2 NeuronCores.

Self-contained: hardcodes shapes for the nn_Architecture_13829794693845 problem.
Sharding: pure data-parallel over batch (B=32 -> 4 sequences/core), params
replicated; no collectives needed. Per-core everything is SBUF-resident.

Device-side design:
- Streams are kept as LayerNorm-normalized z-hats; LN affines fold into the
  consuming weights on the host (W' = diag(s)W, b' = bW + bias) and re-enter
  residual sums via appended diag(s) matmuls + ones-row bias matmuls, so no
  free-axis broadcast is ever needed.
- kq_same: one K-projection serves q and k; raw scores are symmetric.
- Additive -1e32 mask applied only to the diagonal 128x128 block of each
  query chunk; upper blocks never computed (lower-triangular trimming).
- Softmax without max-subtraction (scores are O(10); exp safe in fp32).
- Distance decay: cumE by tensor_tensor_scan; suffix T2 = (Z1-cumE)*pos with
  Z1 = last scan element (exact, so T2 >= 0); dist = exp(0.5*ln(T2*invZ1))
  keeps every ACT op in the natural_log_exp table set (no table reloads).
- zero_pad falls out of Z2 + 1e-30: fully-masked rows scale to exactly 0.
- E2 normalized by invZ2 per-partition before the PE transpose; AV uses the
  transposed a^T with v as the stationary operand.
"""

import numpy as np
import ml_dtypes

B, S, D, H, DFF = 32, 512, 256, 8, 1024
L = 6
DK = D // H          # 32
SCALE = 1.0 / float(np.sqrt(DK))
NEG = -1e32
NCORES = 8
BLOC = B // NCORES   # 4
T = BLOC * S         # 2048 tokens per core
P = 128
KC = D // P          # 2 feature chunks
FC = DFF // P        # 8 ffn chunks
TC = T // P          # 16 token chunks
IC = S // P          # 4 query chunks per sequence
LN_EPS = 1e-5

CFGS = [
    dict(st="y", v="y", strict=False, ffn=True),
    dict(st="y", v="y", strict=False, ffn=True),
    dict(st="x", v="x", strict=False, ffn=False),
    dict(st="x", v="y", strict=True, ffn=True),
    dict(st="x", v="x", strict=False, ffn=False),
    dict(st="x", v="y", strict=True, ffn=True),
]
FFN_LAYERS = [0, 1, 3, 5]
FFN_IDX = {l: i for i, l in enumerate(FFN_LAYERS)}
NF = len(FFN_LAYERS)

BF16NP = ml_dtypes.bfloat16

# packed lower-triangular i-chunk slice layout (widths 128/256/384/512)
WID = [(ic + 1) * P for ic in range(IC)]
OFF = [0, 128, 384, 768]
TRI = 1280
# scores psum layout: no matmul output crosses a 512-col fp32 bank boundary
PSOFF = [0, 128, 512, 1024]
# packed j-chunk layout for a^T (widths 512/384/256/128)
JOFF = [0, 512, 896, 1152]


def _softplus(x):
    return np.log1p(np.exp(-np.abs(x))) + np.maximum(x, 0.0)


# ----------------------------------------------------------------------------
# Host-side preprocessing
# ----------------------------------------------------------------------------

def _prep(inputs):
    f32 = np.float32
    Wk, bk = f32(inputs["Wk"]), f32(inputs["bk"])
    Wv, bv = f32(inputs["Wv"]), f32(inputs["bv"])
    Wo, bo = f32(inputs["Wo"]), f32(inputs["bo"])
    gam = f32(inputs["gammas"]).reshape(L, H)
    l1s, l1b = f32(inputs["ln1_s"]), f32(inputs["ln1_b"])
    l2s, l2b = f32(inputs["ln2_s"]), f32(inputs["ln2_b"])
    W1, b1 = f32(inputs["W1"]), f32(inputs["b1"])
    W2, b2 = f32(inputs["W2"]), f32(inputs["b2"])

    cur = {"y": (np.ones(D, f32), np.zeros(D, f32)),
           "x": (np.ones(D, f32), np.zeros(D, f32))}

    wk_p = np.zeros((P, L * KC * D), BF16NP)
    wv_p = np.zeros((P, L * KC * D), BF16NP)
    wo_p = np.zeros((P, L * KC * D), BF16NP)
    diaga_p = np.zeros((P, L * KC * P), BF16NP)
    diagb_p = np.zeros((P, L * KC * P), BF16NP)
    w1_p = np.zeros((P, NF * KC * DFF), BF16NP)
    w2_p = np.zeros((P, NF * FC * D), BF16NP)
    bk_p = np.zeros((P, L * KC), f32)
    b1_p = np.zeros((P, NF * FC), f32)
    brow_v = np.zeros((1, L * D), BF16NP)
    brow_o = np.zeros((1, L * D), BF16NP)
    brow_2 = np.zeros((1, NF * D), BF16NP)

    for l, cfg in enumerate(CFGS):
        s_f, b_f = cur[cfg["st"]]
        s_fv, b_fv = cur[cfg["v"]]
        Wk_eff = s_f[:, None] * Wk[l]
        bk_eff = b_f @ Wk[l] + bk[l]
        Wv_eff = s_fv[:, None] * Wv[l]
        bv_eff = b_fv @ Wv[l] + bv[l]
        for kc in range(KC):
            j = (l * KC + kc)
            wk_p[:, j * D:(j + 1) * D] = Wk_eff[kc * P:(kc + 1) * P]
            wv_p[:, j * D:(j + 1) * D] = Wv_eff[kc * P:(kc + 1) * P]
            wo_p[:, j * D:(j + 1) * D] = Wo[l][kc * P:(kc + 1) * P]
            diaga_p[:, j * P:(j + 1) * P] = np.diag(s_f[kc * P:(kc + 1) * P])
            diagb_p[:, j * P:(j + 1) * P] = np.diag(l1s[l][kc * P:(kc + 1) * P])
            bk_p[:, j] = bk_eff[kc * P:(kc + 1) * P]
        brow_v[0, l * D:(l + 1) * D] = bv_eff
        brow_o[0, l * D:(l + 1) * D] = bo[l] + b_f

        if cfg["ffn"]:
            fi = FFN_IDX[l]
            W1_eff = l1s[l][:, None] * W1[l]
            b1_eff = l1b[l] @ W1[l] + b1[l]
            for kc in range(KC):
                j = fi * KC + kc
                w1_p[:, j * DFF:(j + 1) * DFF] = W1_eff[kc * P:(kc + 1) * P]
            for kc in range(FC):
                j = fi * FC + kc
                w2_p[:, j * D:(j + 1) * D] = W2[l][kc * P:(kc + 1) * P]
                b1_p[:, j] = b1_eff[kc * P:(kc + 1) * P]
            brow_2[0, fi * D:(fi + 1) * D] = b2[l] + l1b[l]
            cur[cfg["st"]] = (l2s[l], l2b[l])
        else:
            cur[cfg["st"]] = (l1s[l], l1b[l])

    s_x, b_x = cur["x"]
    diagf_p = np.zeros((P, KC * P), BF16NP)
    for kc in range(KC):
        diagf_p[:, kc * P:(kc + 1) * P] = np.diag(s_x[kc * P:(kc + 1) * P])
    brow_f = np.zeros((1, D), BF16NP)
    brow_f[0] = b_x

    gneg = np.repeat((-_softplus(gam)).reshape(1, L * H), P, axis=0).astype(f32)

    idx = np.arange(S, dtype=f32)
    negpos = np.zeros((P, IC * S), np.float16)
    for ic in range(IC):
        rows = idx[ic * P:(ic + 1) * P]
        negpos[:, ic * S:(ic + 1) * S] = -np.abs(rows[:, None] - idx[None, :])

    maskd = np.zeros((P, 2 * P), f32)
    r = np.arange(P)
    maskd[:, 0:P] = np.where(r[None, :] <= r[:, None], 0.0, NEG)      # inclusive
    maskd[:, P:2 * P] = np.where(r[None, :] < r[:, None], 0.0, NEG)   # strict

    shared = dict(
        wk=wk_p, wv=wv_p, wo=wo_p, diaga=diaga_p, diagb=diagb_p,
        diagf=diagf_p, w1=w1_p, w2=w2_p, bk=bk_p, b1=b1_p,
        brow_v=brow_v, brow_o=brow_o, brow_2=brow_2, brow_f=brow_f,
        gneg=gneg, negpos=negpos, maskd=maskd,
    )

    q_embed = np.asarray(inputs["q_embed_data"], f32)
    qa_embed = np.asarray(inputs["qa_embed_data"], f32)

    def shard_T(x, c):
        xt = x[c * BLOC:(c + 1) * BLOC].reshape(T, D).T   # [D, T]
        out = np.zeros((P, KC * T), BF16NP)
        for kc in range(KC):
            out[:, kc * T:(kc + 1) * T] = xt[kc * P:(kc + 1) * P]
        return out

    return [dict(shared, x0t=shard_T(q_embed, c), y0t=shard_T(qa_embed, c))
            for c in range(NCORES)]


# ----------------------------------------------------------------------------
# Device kernel
# ----------------------------------------------------------------------------

_CACHED = {}


def _build():
    import concourse.bass as bass  # noqa: F401
    import concourse.tile as tile
    from concourse import bacc, mybir

    dt = mybir.dt
    F32, BF, F16 = dt.float32, dt.bfloat16, dt.float16
    Alu = mybir.AluOpType
    Act = mybir.ActivationFunctionType
    AX = mybir.AxisListType.X

    nc = bacc.Bacc("TRN2", target_bir_lowering=False, debug=False)

    dram = {}

    def din(name, shape, dtype):
        dram[name] = nc.dram_tensor(name, list(shape), dtype, kind="ExternalInput")

    din("x0t", (P, KC * T), BF)
    din("y0t", (P, KC * T), BF)
    din("wk", (P, L * KC * D), BF)
    din("wv", (P, L * KC * D), BF)
    din("wo", (P, L * KC * D), BF)
    din("diaga", (P, L * KC * P), BF)
    din("diagb", (P, L * KC * P), BF)
    din("diagf", (P, KC * P), BF)
    din("w1", (P, NF * KC * DFF), BF)
    din("w2", (P, NF * FC * D), BF)
    din("bk", (P, L * KC), F32)
    din("b1", (P, NF * FC), F32)
    din("brow_v", (1, L * D), BF)
    din("brow_o", (1, L * D), BF)
    din("brow_2", (1, NF * D), BF)
    din("brow_f", (1, D), BF)
    din("gneg", (P, L * H), F32)
    din("negpos", (P, IC * S), F16)
    din("maskd", (P, 2 * P), F32)
    out_d = nc.dram_tensor("out", [T, D], F32, kind="ExternalOutput")

    with tile.TileContext(nc) as tc:
        with (
            tc.tile_pool(name="const", bufs=1) as cpool,
            tc.tile_pool(name="work", bufs=1) as wpool,
            tc.tile_pool(name="strm_o", bufs=2) as spool_o,
            tc.tile_pool(name="strm_2", bufs=2) as spool_2,
            tc.tile_pool(name="attn2", bufs=2) as apool2,
            tc.tile_pool(name="attn1", bufs=1) as apool1,
            tc.tile_pool(name="wts", bufs=2) as wts,
            tc.tile_pool(name="zcol", bufs=3) as zpool,
            tc.tile_pool(name="ppool", bufs=6, space="PSUM") as ppool,
            tc.tile_pool(name="psmall", bufs=1, space="PSUM") as psmall,
        ):
            def load(name, shape, dtype, tag):
                t = cpool.tile(list(shape), dtype, tag=tag)
                nc.sync.dma_start(t[:], dram[name][:])
                return t

            x0t = load("x0t", (P, KC * T), BF, "x0t")
            y0t = load("y0t", (P, KC * T), BF, "y0t")
            diagf = load("diagf", (P, KC * P), BF, "diagf")
            bkt = load("bk", (P, L * KC), F32, "bk")
            b1t = load("b1", (P, NF * FC), F32, "b1")
            brow_v = load("brow_v", (1, L * D), BF, "brow_v")
            brow_o = load("brow_o", (1, L * D), BF, "brow_o")
            brow_2 = load("brow_2", (1, NF * D), BF, "brow_2")
            brow_f = load("brow_f", (1, D), BF, "brow_f")
            gneg = load("gneg", (P, L * H), F32, "gneg")
            negpos = load("negpos", (P, IC * S), F16, "negpos")
            maskd = load("maskd", (P, 2 * P), F32, "maskd")

            ones1 = cpool.tile([1, P], BF, tag="ones1")
            nc.vector.memset(ones1[:], 1.0)
            onesP = cpool.tile([P, P], BF, tag="onesP")
            nc.gpsimd.memset(onesP[:], 1.0)
            ident = cpool.tile([P, P], BF, tag="ident")
            nc.gpsimd.affine_select(
                ident[:], onesP[:], pattern=[[-1, P]], compare_op=Alu.is_equal,
                fill=0.0, base=0, channel_multiplier=1,
            )
            eps_t = cpool.tile([P, 1], F32, tag="eps")
            nc.vector.memset(eps_t[:], LN_EPS)
            lneps_t = cpool.tile([P, 1], F32, tag="lneps")
            nc.vector.memset(lneps_t[:], 1e-30)

            # ------------------------------------------------------------
            def attention_head(l, b, h, moff, qkT, v_tok, av_ps):
                mc, prow = h // 4, (h % 4) * 32

                def qk(lo, hi):
                    return qkT[prow:prow + 32, mc * T + b * S + lo:mc * T + b * S + hi]

                sps = []
                for ic in range(IC):
                    W = WID[ic]
                    sp = ppool.tile([P, W], F32, tag="sps")
                    sps.append(sp)
                    nc.tensor.matmul(
                        sp[:], qk(ic * P, (ic + 1) * P), qk(0, W),
                        start=True, stop=True, tile_position=(prow, 0),
                    )
                    nc.vector.tensor_tensor(
                        sp[:, ic * P:W], sp[:, ic * P:W],
                        maskd[:, moff:moff + P], op=Alu.add,
                    )

                zc = zpool.tile([P, 8], F32, tag="zc")
                E = apool2.tile([P, TRI], F32, tag="E")
                for ic in range(IC):
                    W = WID[ic]
                    nc.scalar.activation(
                        E[:, OFF[ic]:OFF[ic] + W], sps[ic][:],
                        Act.Exp, scale=SCALE, accum_out=zc[:, ic:ic + 1],
                    )
                nc.vector.tensor_scalar(zc[:, 4:8], zc[:, 0:4], 1.0 + 2e-7, 1e-30,
                                        op0=Alu.mult, op1=Alu.add)
                nc.vector.reciprocal(zc[:, 4:8], zc[:, 4:8])

                cumE = apool1.tile([P, TRI], F32, tag="cumE")
                for ic in range(IC):
                    W = WID[ic]
                    nc.vector.tensor_tensor_scan(
                        cumE[:, OFF[ic]:OFF[ic] + W], E[:, OFF[ic]:OFF[ic] + W],
                        E[:, OFF[ic]:OFF[ic] + W], 0.0, op0=Alu.add, op1=Alu.bypass,
                    )
                t2 = apool1.tile([P, TRI], F32, tag="t2")
                for ic in range(IC):
                    W = WID[ic]
                    nc.vector.scalar_tensor_tensor(
                        t2[:, OFF[ic]:OFF[ic] + W], cumE[:, OFF[ic]:OFF[ic] + W],
                        cumE[:, OFF[ic] + W - 1:OFF[ic] + W],
                        negpos[:, ic * S:ic * S + W],
                        op0=Alu.subtract, op1=Alu.mult,
                    )
                lt = apool1.tile([P, TRI], F16, tag="lt")
                for ic in range(IC):
                    W = WID[ic]
                    nc.scalar.activation(
                        lt[:, OFF[ic]:OFF[ic] + W], t2[:, OFF[ic]:OFF[ic] + W],
                        Act.Ln, bias=lneps_t[:, 0:1], scale=zc[:, 4 + ic:5 + ic],
                    )
                dist = apool1.tile([P, TRI], F16, tag="dist")
                nc.scalar.activation(dist[:], lt[:], Act.Exp, scale=0.5)
                eff = apool1.tile([P, TRI], F16, tag="eff")
                nc.scalar.activation(eff[:], dist[:], Act.Exp,
                                     scale=gneg[:, l * H + h:l * H + h + 1])
                u = apool1.tile([P, TRI], F32, tag="u")
                for ic in range(IC):
                    W = WID[ic]
                    nc.vector.scalar_tensor_tensor(
                        u[:, OFF[ic]:OFF[ic] + W], eff[:, OFF[ic]:OFF[ic] + W],
                        1e-5, sps[ic][:],
                        op0=Alu.max, op1=Alu.mult,
                    )
                e2 = apool2.tile([P, TRI], BF, tag="e2")
                z2 = zpool.tile([P, 8], F32, tag="z2")
                for ic in range(IC):
                    W = WID[ic]
                    nc.scalar.activation(
                        e2[:, OFF[ic]:OFF[ic] + W], u[:, OFF[ic]:OFF[ic] + W],
                        Act.Exp, scale=SCALE, accum_out=z2[:, ic:ic + 1],
                    )
                nc.vector.tensor_scalar(z2[:, 4:8], z2[:, 0:4], 1e-30, None,
                                        op0=Alu.add)
                nc.vector.reciprocal(z2[:, 4:8], z2[:, 4:8])
                for ic in range(IC):
                    W = WID[ic]
                    nc.vector.tensor_scalar(
                        e2[:, OFF[ic]:OFF[ic] + W], e2[:, OFF[ic]:OFF[ic] + W],
                        z2[:, 4 + ic:5 + ic], None, op0=Alu.mult,
                    )
                aT_sb = apool2.tile([P, TRI], BF, tag="aT_sb")
                for jc in range(IC):
                    wj = S - jc * P
                    aT_ps = psmall.tile([P, 512], BF, tag="aT")
                    for ic in range(jc, IC):
                        nc.tensor.matmul(
                            aT_ps[:, (ic - jc) * P:(ic - jc + 1) * P],
                            e2[:, OFF[ic] + jc * P:OFF[ic] + (jc + 1) * P],
                            ident[:], is_transpose=True, start=(ic == jc),
                            stop=True, skip_group_check=True,
                        )
                    nc.vector.tensor_copy(aT_sb[:, JOFF[jc]:JOFF[jc] + wj],
                                          aT_ps[:, 0:wj])
                    nc.tensor.matmul(
                        av_ps[prow:prow + 32, jc * P:S],
                        v_tok[:, (b * IC + jc) * D + h * 32:(b * IC + jc) * D + h * 32 + 32],
                        aT_sb[:, JOFF[jc]:JOFF[jc] + wj],
                        start=(jc == 0), stop=(jc == IC - 1), skip_group_check=True,
                        tile_position=(0, prow),
                    )

            # ------------------------------------------------------------
            def proj_residual_ln(l, which, featT, wmat, nkc, widx, zfeed,
                                 diag, brow, out_pool):
                """r = featT.T@W + zfeed.T@diag + brow ; LN -> zhatT bf16."""
                zh_tok = wpool.tile([P, TC * D], BF, tag="zh_tok")
                stats = zpool.tile([P, 64], F32, tag="stats")
                scratch = wpool.tile([P, 4 * D], F16, tag="sq_scratch")
                r_sb = wpool.tile([P, TC * D], F16, tag="r_sb")

                for g in range(4):
                    for q in range(4):
                        tcx = g * 4 + q
                        ps = ppool.tile([P, D], F32, tag="sps")
                        for kc in range(nkc):
                            nc.tensor.matmul(
                                ps[:],
                                featT[:, kc * T + tcx * P:kc * T + (tcx + 1) * P],
                                wmat[:, kc * D:(kc + 1) * D],
                                start=(kc == 0), stop=False, skip_group_check=True,
                            )
                        for kc in range(KC):
                            nc.tensor.matmul(
                                ps[:, kc * P:(kc + 1) * P],
                                zfeed[:, kc * T + tcx * P:kc * T + (tcx + 1) * P],
                                diag[:, kc * P:(kc + 1) * P],
                                start=False, stop=False, skip_group_check=True,
                            )
                        nc.tensor.matmul(
                            ps[:], ones1[:],
                            brow[:, widx * D:(widx + 1) * D],
                            start=False, stop=True, skip_group_check=True,
                        )
                        nc.vector.tensor_copy(
                            r_sb[:, tcx * D:(tcx + 1) * D], ps[:])
                    rg = r_sb[:, g * 4 * D:(g + 1) * 4 * D].rearrange(
                        "p (q d) -> p q d", d=D)
                    nc.vector.tensor_reduce(stats[:, g * 4:(g + 1) * 4], rg,
                                            axis=AX, op=Alu.add)
                    nc.vector.tensor_tensor(
                        scratch[:], r_sb[:, g * 4 * D:(g + 1) * 4 * D],
                        r_sb[:, g * 4 * D:(g + 1) * 4 * D], op=Alu.mult)
                    sg = scratch[:].rearrange("p (q d) -> p q d", d=D)
                    nc.vector.tensor_reduce(stats[:, 16 + g * 4:16 + (g + 1) * 4],
                                            sg, axis=AX, op=Alu.add)

                # mu = s/D ; var = sq/D - mu^2 ; invstd = exp(-0.5*ln(var+eps))
                nc.vector.tensor_scalar(stats[:, 32:48], stats[:, 0:16], 1.0 / D,
                                        None, op0=Alu.mult)
                nc.vector.tensor_tensor(stats[:, 48:64], stats[:, 32:48],
                                        stats[:, 32:48], op=Alu.mult)
                nc.vector.scalar_tensor_tensor(
                    stats[:, 48:64], stats[:, 16:32], 1.0 / D, stats[:, 48:64],
                    op0=Alu.mult, op1=Alu.subtract)
                nc.scalar.activation(stats[:, 48:64], stats[:, 48:64], Act.Ln,
                                     bias=eps_t[:, 0:1], scale=1.0)
                nc.scalar.activation(stats[:, 48:64], stats[:, 48:64], Act.Exp,
                                     scale=-0.5)

                for tcx in range(TC):
                    nc.vector.tensor_scalar(
                        zh_tok[:, tcx * D:(tcx + 1) * D],
                        r_sb[:, tcx * D:(tcx + 1) * D],
                        stats[:, 32 + tcx:33 + tcx], stats[:, 48 + tcx:49 + tcx],
                        op0=Alu.subtract, op1=Alu.mult,
                    )

                zhT = out_pool.tile([P, KC * T], BF, tag=f"zhT_{which}")
                for kc in range(KC):
                    for g in range(4):
                        tps = ppool.tile([P, 512], BF, tag="sps")
                        for q in range(4):
                            tcx = g * 4 + q
                            nc.tensor.matmul(
                                tps[:, q * P:(q + 1) * P],
                                zh_tok[:, tcx * D + kc * P:tcx * D + (kc + 1) * P],
                                ident[:], is_transpose=True, start=True, stop=True,
                                skip_group_check=True,
                            )
                        nc.vector.tensor_copy(
                            zhT[:, kc * T + g * 512:kc * T + (g + 1) * 512], tps[:])
                return zhT

            # ------------------------------------------------------------
            cur = {"y": y0t, "x": x0t}

            for l, cfg in enumerate(CFGS):
                zq = cur[cfg["st"]]
                zv = cur[cfg["v"]]
                moff = P if cfg["strict"] else 0

                wkl = wts.tile([P, KC * D], BF, tag="wkl")
                nc.sync.dma_start(wkl[:], dram["wk"][:, l * KC * D:(l + 1) * KC * D])
                wvl = wts.tile([P, KC * D], BF, tag="wvl")
                nc.sync.dma_start(wvl[:], dram["wv"][:, l * KC * D:(l + 1) * KC * D])
                wol = wts.tile([P, KC * D], BF, tag="wol")
                nc.sync.dma_start(wol[:], dram["wo"][:, l * KC * D:(l + 1) * KC * D])
                dal = wts.tile([P, KC * P], BF, tag="dal")
                nc.sync.dma_start(dal[:], dram["diaga"][:, l * KC * P:(l + 1) * KC * P])

                # A: K projection -> qkT feature-major [P, KC*T]
                qkT = wpool.tile([P, KC * T], BF, tag="qkT")
                for mc in range(KC):
                    for nt in range(T // 512):
                        ps = ppool.tile([P, 512], F32, tag="sps")
                        for kc in range(KC):
                            nc.tensor.matmul(
                                ps[:],
                                wkl[:, kc * D + mc * P:kc * D + (mc + 1) * P],
                                zq[:, kc * T + nt * 512:kc * T + (nt + 1) * 512],
                                start=(kc == 0), stop=(kc == KC - 1),
                            )
                        nc.vector.tensor_scalar(
                            qkT[:, mc * T + nt * 512:mc * T + (nt + 1) * 512],
                            ps[:], bkt[:, l * KC + mc:l * KC + mc + 1], None,
                            op0=Alu.add,
                        )

                # B: V projection -> v_tok token-major [P, TC*D]
                v_tok = wpool.tile([P, TC * D], BF, tag="v_tok")
                for tcx in range(TC):
                    ps = ppool.tile([P, D], F32, tag="sps")
                    for kc in range(KC):
                        nc.tensor.matmul(
                            ps[:], zv[:, kc * T + tcx * P:kc * T + (tcx + 1) * P],
                            wvl[:, kc * D:(kc + 1) * D],
                            start=(kc == 0), stop=False,
                        )
                    nc.tensor.matmul(ps[:], ones1[:], brow_v[:, l * D:(l + 1) * D],
                                     start=False, stop=True)
                    nc.vector.tensor_copy(v_tok[:, tcx * D:(tcx + 1) * D], ps[:])

                # C: attention
                attnT = wpool.tile([P, KC * T], BF, tag="attnT")
                for b in range(BLOC):
                    for hg in range(2):
                        av_ps = psmall.tile([P, S], F32, tag="av")
                        for hh in range(4):
                            attention_head(l, b, hg * 4 + hh, moff, qkT, v_tok, av_ps)
                        nc.vector.tensor_copy(
                            attnT[:, hg * T + b * S:hg * T + (b + 1) * S], av_ps[:])

                # D: Wo + residual + LN1
                zh1 = proj_residual_ln(l, "o", attnT, wol, KC, l, zq, dal,
                                       brow_o, spool_o)

                if cfg["ffn"]:
                    fi = FFN_IDX[l]
                    w1l = wts.tile([P, KC * DFF], BF, tag="w1l")
                    nc.sync.dma_start(w1l[:], dram["w1"][:, fi * KC * DFF:(fi + 1) * KC * DFF])
                    w2l = wts.tile([P, FC * D], BF, tag="w2l")
                    nc.sync.dma_start(w2l[:], dram["w2"][:, fi * FC * D:(fi + 1) * FC * D])
                    dbl = wts.tile([P, KC * P], BF, tag="dbl")
                    nc.sync.dma_start(dbl[:], dram["diagb"][:, l * KC * P:(l + 1) * KC * P])
                    hT = wpool.tile([P, FC * T], BF, tag="hT")
                    for mc in range(FC):
                        for nt in range(T // 512):
                            ps = ppool.tile([P, 512], F32, tag="sps")
                            for kc in range(KC):
                                nc.tensor.matmul(
                                    ps[:],
                                    w1l[:, kc * DFF + mc * P:kc * DFF + (mc + 1) * P],
                                    zh1[:, kc * T + nt * 512:kc * T + (nt + 1) * 512],
                                    start=(kc == 0), stop=(kc == KC - 1),
                                )
                            nc.vector.tensor_scalar(
                                hT[:, mc * T + nt * 512:mc * T + (nt + 1) * 512],
                                ps[:], b1t[:, fi * FC + mc:fi * FC + mc + 1], 0.0,
                                op0=Alu.add, op1=Alu.max,
                            )
                    zh2 = proj_residual_ln(l, "2", hT, w2l, FC, fi, zh1, dbl,
                                           brow_2, spool_2)
                    cur[cfg["st"]] = zh2
                else:
                    cur[cfg["st"]] = zh1

                if l == 1:
                    yfin = cpool.tile([P, KC * T], BF, tag="y0t")
                    nc.vector.tensor_copy(yfin[:], cur["y"][:])
                    cur["y"] = yfin

            # final affine: out = s*zx + b
            zx = cur["x"]
            out_dr = out_d.rearrange("(t p) d -> p t d", p=P)
            for g in range(4):
                out_sb = cpool.tile([P, 4 * D], F32, tag="x0t")
                for q in range(4):
                    tcx = g * 4 + q
                    ps = ppool.tile([P, D], F32, tag="sps")
                    for kc in range(KC):
                        nc.tensor.matmul(
                            ps[:, kc * P:(kc + 1) * P],
                            zx[:, kc * T + tcx * P:kc * T + (tcx + 1) * P],
                            diagf[:, kc * P:(kc + 1) * P],
                            start=(kc == 0), stop=False, skip_group_check=True,
                        )
                    nc.tensor.matmul(ps[:], ones1[:], brow_f[:],
                                     start=False, stop=True, skip_group_check=True)
                    nc.vector.tensor_copy(out_sb[:, q * D:(q + 1) * D], ps[:])
                nc.sync.dma_start(
                    out_dr[:, g * 4:(g + 1) * 4, :],
                    out_sb[:].rearrange("p (t d) -> p t d", d=D))

    nc.compile()
    _dedupe_act_table_loads(nc, mybir)
    return nc


def _dedupe_act_table_loads(nc, mybir):
    """All ACT funcs here are Exp/Ln; one load of natural_log_exp_and_others
    (which contains both) replaces the compiler's per-switch reloads."""
    from concourse.hw_specs import get_activation_tables
    tables = list(get_activation_tables(nc.m.arch).items())
    target = None
    for idx, (name, funcs) in enumerate(tables):
        A = mybir.ActivationFunctionType
        if A.Exp in funcs and A.Ln in funcs:
            target = idx
            break
    assert target is not None
    first = True
    for blk in nc.m.functions[0].blocks:
        keep = []
        for inst in blk.instructions:
            if isinstance(inst, mybir.InstLoadActFuncSet):
                if first:
                    inst.act_func_set_id = target
                    keep.append(inst)
                    first = False
                continue
            keep.append(inst)
        blk.set_instructions_from_list(keep) if hasattr(blk, 'set_instructions_from_list') else None
        if not hasattr(blk, 'set_instructions_from_list'):
            # mutate in place
            insts = blk.instructions
            while len(insts):
                insts.pop()
            for k in keep:
                insts.append(k)
    return nc


def kernel(**inputs):
    from concourse.bass_utils import run_bass_kernel_spmd

    if "nc" not in _CACHED:
        _CACHED["nc"] = _build()
    nc = _CACHED["nc"]

    in_maps = _prep(inputs)
    res = run_bass_kernel_spmd(nc, in_maps, core_ids=list(range(NCORES)))
    outs = [np.asarray(res.results[c]["out"]).reshape(BLOC, S, D)
            for c in range(NCORES)]
    return np.concatenate(outs, axis=0).astype(np.float32)


if __name__ == "__main__":
    import reference
    inp = {k: np.asarray(v) for k, v in reference.setup_inputs().items()}
    out = kernel(**inp)
    print("out", out.shape, out.dtype)

